# revision 44
# baseline (speedup 1.0000x reference)
"""Trainium2 Bass kernel for Bahdanau-style attention scoring.

Reference computation (per batch b):
    h_proj = hidden @ Wh.T + b_attn                  # [D]
    c_proj[s] = Wc @ context[b, s]                   # [S, D]
    scores[s] = v . tanh(h_proj + c_proj[s])         # [S]
    out[b] = softmax(where(mask==0, -inf, scores))   # [S]

Strategy: data-parallel over batch B across 8 NeuronCores (4 batches/core).
The roofline is the c_proj matmul: 1024 [128x128]x[128x512] fp16 matmuls per
core (~222us at the ~216ns/MM N=512 pace). The context is shipped to device
DRAM pre-cast to fp16 (32 MiB/core, ~100us of DMA — same values the SWDGE
cast-on-DMA datapath would produce, at half the HBM traffic), so the
TensorE matmul stream is the single roofline and everything else must stay
off it.

So unlike the usual [d, s] layout, c_proj is computed TRANSPOSED, [s, d]:
the context tile [e=128, s=128] is the stationary operand and WcT [e=128,
d=512] is the moving operand, giving PSUM tiles [s=128, d=512]. With d on
the free axis the v-dot after tanh is a free-axis mul+reduce on VectorE
instead of 128 extra TensorE mat-vec matmuls (which cost ~28us of PE in the
[d, s] layout).

h_proj is folded into the context on the host, exactly: Wc has full row
rank, so delta_b = Wc^T (Wc Wc^T)^{-1} h_proj_b satisfies
Wc (x + delta_b) = c_proj + h_proj_b, and the shard prep adds delta_b[e]
to batch b's context rows. PSUM then holds tanh's full argument directly
(no per-tile broadcast bias add on the [s, d] free axis, which VectorE
would otherwise have to do at fp32 pace).

Per (b, s-tile of 128):
  - 8 accumulating matmuls (e-chunks) -> PSUM y.T [s=128, d=512]
  - ScalarE tanh (PSUM -> SBUF fp16)
  - VectorE (sim * v_bcast) at fp16 2x pace, then free-axis reduce_sum ->
    one column of the per-batch scores tile [128, 32]  (s = tile*128 + p)
Per b (deferred one s-window so TensorE never waits on the chain):
  - ScalarE exp (no max subtraction: |scores| < ~35, far under f32 range),
    VectorE mask-multiply and row-sum, TensorE ones-matmul replicates the
    total over all partitions, VectorE reciprocal, TensorE transpose to
    [32, 128] row-major, VectorE scale-on-copy, DMA out.

DMA queues: gpsimd/SWDGE carries the big context loads, sync/scalar HWDGE
split the wcT preamble between them and carry the outputs. While the ~2MB
preamble (wcT + window 0) fills, ~72 small junk warm-up matmuls bring the PE HAM
clock-gate to 8/8, and window 0's contraction is swept g-outermost across
all four PSUM banks so each matmul only needs the (wcT, ctx) chunk pair
that has already arrived — the PE computes through the fill instead of
waiting for it.
"""

import numpy as np

import concourse.bacc as bacc
import concourse.mybir as mybir
from concourse.tile import TileContext
from concourse.bass_utils import run_bass_kernel_spmd

B, S, E, D = 32, 4096, 1024, 512
NCORES = 8
BL = B // NCORES  # batches per core

F32 = mybir.dt.float32
F16 = mybir.dt.float16


def build_graph(bl=BL, s=S, e=E, d=D, ncores=NCORES):
    """Build the per-core Bass graph. All cores run the same graph (SPMD)."""
    G = e // 128        # e-chunks (contraction passes per tile)
    SW = 512            # s-window per context DMA (4 s-tiles)
    NSW = s // SW       # s-windows per batch
    TPW = SW // 128     # s-tiles per window
    NT = s // 128       # s-tiles per batch (scores columns)
    AF = mybir.ActivationFunctionType

    nc = bacc.Bacc("TRN2", target_bir_lowering=False, debug=False,
                   num_devices=ncores)

    ctxT = nc.dram_tensor("ctxT", [bl, e, s], F16, kind="ExternalInput")
    wcT = nc.dram_tensor("wcT", [128, G, d], F16, kind="ExternalInput")
    vb = nc.dram_tensor("vb", [128, d], F16, kind="ExternalInput")
    eye = nc.dram_tensor("eye", [128, 128], F32, kind="ExternalInput")
    maskT = nc.dram_tensor("maskT", [128, bl * NT], F32, kind="ExternalInput")
    out = nc.dram_tensor("out", [bl, s], F32, kind="ExternalOutput")

    ctx_r = ctxT.ap().rearrange("b (g p) s -> b p g s", p=128)
    out_r = out.ap().rearrange("b (t x) -> b t x", x=128)

    with TileContext(nc) as tc:
        with (
            tc.tile_pool(name="const", bufs=1) as cpool,
            tc.tile_pool(name="ctx", bufs=6) as ctx_pool,
            tc.tile_pool(name="sim", bufs=4) as sim_pool,
            tc.tile_pool(name="prod", bufs=4) as prod_pool,
            tc.tile_pool(name="sc", bufs=2) as sc_pool,
            tc.tile_pool(name="small", bufs=2) as small_pool,
            tc.tile_pool(name="pc", bufs=5, space="PSUM") as pc_pool,
            tc.tile_pool(name="tail", bufs=1, space="PSUM") as tail_pool,
            tc.tile_pool(name="warm", bufs=1, space="PSUM") as warm_pool,
        ):
            # ---- constants / preamble ------------------------------------
            # per-g wcT chunks alternating between the two HWDGE queues:
            # g0 lands first so the first matmul group can start while the
            # later chunks stream in behind it.
            wct_sb = cpool.tile([128, G, d], F16, tag="wct")
            h = G // 2
            for g in range(0, G, 2):
                nc.scalar.dma_start(out=wct_sb[:, g, :], in_=wcT.ap()[:, g, :])
                nc.sync.dma_start(out=wct_sb[:, g + 1, :],
                                  in_=wcT.ap()[:, g + 1, :])
            ones128 = cpool.tile([128, 128], F32, tag="ones128")
            nc.vector.memset(ones128[:], 1.0)
            vb_sb = cpool.tile([128, d], F16, tag="vb")
            nc.sync.dma_start(out=vb_sb[:], in_=vb.ap())

            # eye/maskT are not needed until the first softmax tail
            # (~70us in); declared here, loaded after window 1 is queued on
            # the gpsimd queue so the HWDGE rings stay clear for wcT, whose
            # last chunk gates group 0.
            eye_sb = cpool.tile([128, 128], F32, tag="eye")
            maskt_sb = cpool.tile([128, bl * NT], F32, tag="maskt")

            def load_tail_consts():
                nc.gpsimd.dma_start(out=eye_sb[:], in_=eye.ap())
                nc.gpsimd.dma_start(out=maskt_sb[:], in_=maskT.ap())

            # PE warm-up: junk matmuls while the first context chunks
            # stream in, so the HAM clock-gate is warming before the real
            # stream starts (cold matmuls run at half rate).
            junk = cpool.tile([128, 128], F16, tag="junk")
            nc.vector.memset(junk[:], 0.0)
            warm_ps = warm_pool.tile([128, 128], F32, tag="warm")
            for _ in range(72):
                nc.tensor.matmul(warm_ps[:, 0:64], lhsT=junk[:],
                                 rhs=junk[:, 0:64], start=True, stop=True)

            # ---- main loop ------------------------------------------------
            pend = None  # deferred softmax tail of the previous batch

            def tail(b, scores, part=None):
                # scores [128, NT]: s = t*128 + p.  exp -> mask -> row sums
                rsum = small_pool.tile([128, 1], F32, tag="rsum")
                if part is None:
                    erow = small_pool.tile([128, NT], F32, tag="erow")
                    nc.scalar.activation(erow[:], scores[:], AF.Exp)
                    em = small_pool.tile([128, NT], F32, tag="em")
                    nc.vector.tensor_mul(em[:], erow[:],
                                         maskt_sb[:, b * NT:(b + 1) * NT])
                    nc.vector.reduce_sum(rsum[:], em[:],
                                         axis=mybir.AxisListType.X)
                else:
                    # columns 0:cc were exp'd/masked/summed a window ago
                    erow, em, rsa = part
                    cc = (NSW - 1) * TPW
                    nc.scalar.activation(erow[:, cc:NT], scores[:, cc:NT],
                                         AF.Exp)
                    nc.vector.tensor_mul(em[:, cc:NT], erow[:, cc:NT],
                                         maskt_sb[:, b * NT + cc:(b + 1) * NT])
                    rsb = small_pool.tile([128, 1], F32, tag="rsb")
                    nc.vector.reduce_sum(rsb[:], em[:, cc:NT],
                                         axis=mybir.AxisListType.X)
                    nc.vector.tensor_add(rsum[:], rsa[:], rsb[:])
                # total over partitions, replicated to every partition
                tot_ps = tail_pool.tile([128, 1], F32, tag="tot")
                nc.tensor.matmul(tot_ps[:], lhsT=ones128[:], rhs=rsum[:],
                                 start=True, stop=True)
                rec = small_pool.tile([128, 1], F32, tag="rec")
                nc.vector.reciprocal(rec[:], tot_ps[:])
                # transpose to row-major [t=32, x=128] and scale on the copy
                et_ps = tail_pool.tile([NT, 128], F32, tag="et")
                nc.tensor.transpose(et_ps[:], em[:], eye_sb[:])
                orow = small_pool.tile([NT, 128], F32, tag="orow")
                nc.vector.tensor_scalar_mul(orow[:], et_ps[:], rec[0:NT, :])
                hh = NT // 2
                nc.sync.dma_start(out=out_r[b, 0:hh], in_=orow[0:hh, :])
                nc.scalar.dma_start(out=out_r[b, hh:NT], in_=orow[hh:NT, :])

            def chain(pc, scores, st, split=False):
                # tanh -> (sim * v) -> free-axis sum -> scores column
                sim = sim_pool.tile([128, 512], F16, tag="sim")
                prod = prod_pool.tile([128, 512], F16, tag="prod")
                if split:
                    # halve the chain so ScalarE/VectorE pipeline the
                    # fully-exposed serial tail after the last matmul group
                    s2 = small_pool.tile([128, 2], F32, tag="s2")
                    for q in range(2):
                        cut = slice(q * 256, (q + 1) * 256)
                        nc.scalar.activation(sim[:, cut], pc[:, cut], AF.Tanh)
                        nc.vector.tensor_mul(prod[:, cut], sim[:, cut],
                                             vb_sb[:, cut])
                        nc.vector.reduce_sum(s2[:, q:q + 1], prod[:, cut],
                                             axis=mybir.AxisListType.X)
                    nc.vector.tensor_add(scores[:, st:st + 1],
                                         s2[:, 0:1], s2[:, 1:2])
                else:
                    nc.scalar.activation(sim[:], pc[:], AF.Tanh)
                    nc.vector.tensor_mul(prod[:], sim[:], vb_sb[:])
                    nc.vector.reduce_sum(scores[:, st:st + 1], prod[:],
                                         axis=mybir.AxisListType.X)

            part = None  # pre-computed exp/mask/rowsum of the last batch

            for b in range(bl):
                scores = sc_pool.tile([128, NT], F32, tag="scores")
                for sw in range(NSW):
                    ctx_slice = ctx_r[b, :, :, sw * SW:(sw + 1) * SW]
                    ctx_t = ctx_pool.tile([128, G, SW], F16, tag="ctx")
                    if b == 0 and sw == 0:
                        # pipe-fill: per-g chunk DMAs, and the contraction
                        # swept g-outer across all 4 PSUM banks — each
                        # matmul needs only the (wcT, ctx) chunk pair g, so
                        # the PE computes through the whole 2MB preamble
                        # fill instead of waiting for it to finish.
                        for g in range(G):
                            nc.gpsimd.dma_start(
                                out=ctx_t[:, g, :], in_=ctx_slice[:, g, :])
                        pcs = [pc_pool.tile([128, 512], F32, tag="pc",
                                            name=f"pc0_{t}")
                               for t in range(TPW)]
                        for g in range(G):
                            for t in range(TPW):
                                nc.tensor.matmul(
                                    pcs[t][:],
                                    lhsT=ctx_t[:, g, t * 128:(t + 1) * 128],
                                    rhs=wct_sb[:, g, :],
                                    start=(g == 0), stop=(g == G - 1),
                                )
                        for t in range(TPW):
                            chain(pcs[t], scores, t)
                        continue
                    if b == 0 and sw == 1:
                        nc.gpsimd.dma_start(
                            out=ctx_t[:, 0:h, :], in_=ctx_slice[:, 0:h, :])
                        nc.gpsimd.dma_start(
                            out=ctx_t[:, h:G, :], in_=ctx_slice[:, h:G, :])
                    else:
                        # 1MB fp16 read per window
                        nc.gpsimd.dma_start(out=ctx_t[:], in_=ctx_slice)
                        if b == 0 and sw == 2:
                            load_tail_consts()
                    for t in range(TPW):
                        st = sw * TPW + t
                        if b == bl - 1 and st == NT - 1:
                            # final tile: two d-half matmul groups, so the
                            # first half's tanh/dot chain runs while the
                            # second half's matmuls are still on the PE —
                            # this chain is the only fully-exposed one.
                            s2 = small_pool.tile([128, 2], F32, tag="s2")
                            for q in range(2):
                                cut = slice(q * 256, (q + 1) * 256)
                                pch = pc_pool.tile([128, 512], F32,
                                                   tag="pc", name=f"pch{q}")
                                for g in range(G):
                                    nc.tensor.matmul(
                                        pch[:, cut],
                                        lhsT=ctx_t[:, g,
                                                   t * 128:(t + 1) * 128],
                                        rhs=wct_sb[:, g, cut],
                                        start=(g == 0), stop=(g == G - 1),
                                    )
                                sim = sim_pool.tile([128, 512], F16,
                                                    tag="sim",
                                                    name=f"simh{q}")
                                nc.scalar.activation(sim[:, cut],
                                                     pch[:, cut], AF.Tanh)
                                prod = prod_pool.tile([128, 512], F16,
                                                      tag="prod",
                                                      name=f"prodh{q}")
                                nc.vector.tensor_mul(prod[:, cut],
                                                     sim[:, cut],
                                                     vb_sb[:, cut])
                                nc.vector.reduce_sum(
                                    s2[:, q:q + 1], prod[:, cut],
                                    axis=mybir.AxisListType.X)
                            nc.vector.tensor_add(scores[:, st:st + 1],
                                                 s2[:, 0:1], s2[:, 1:2])
                            continue
                        pc = pc_pool.tile([128, 512], F32, tag="pc")
                        for g in range(G):
                            nc.tensor.matmul(
                                pc[:],
                                lhsT=ctx_t[:, g, t * 128:(t + 1) * 128],
                                rhs=wct_sb[:, g, :],
                                start=(g == 0), stop=(g == G - 1),
                            )
                        chain(pc, scores, st)
                    # batch b-1's softmax tail goes out after batch b's
                    # first window is queued, so TensorE's tiny tail ops
                    # never make it wait on the Scalar/Vector chain.
                    if sw == 0 and pend is not None:
                        tail(*pend)
                        pend = None
                    if b == bl - 1 and sw == NSW - 2:
                        # pre-exp/mask/rowsum the 28 finished columns of the
                        # final batch so its exposed tail chain is short
                        erow = small_pool.tile([128, NT], F32, tag="erow")
                        em = small_pool.tile([128, NT], F32, tag="em")
                        rsa = small_pool.tile([128, 1], F32, tag="rsa")
                        cc = (NSW - 1) * TPW
                        nc.scalar.activation(erow[:, 0:cc], scores[:, 0:cc],
                                             AF.Exp)
                        nc.vector.tensor_mul(
                            em[:, 0:cc], erow[:, 0:cc],
                            maskt_sb[:, b * NT:b * NT + cc])
                        nc.vector.reduce_sum(rsa[:], em[:, 0:cc],
                                             axis=mybir.AxisListType.X)
                        part = (erow, em, rsa)
                pend = (b, scores)

            tail(*pend, part=part)

    nc.compile()
    return nc


def shard_inputs(hidden, context, mask, W_attn, b_attn, v,
                 bl=BL, s=S, e=E, d=D, ncores=NCORES):
    """Host-side shard + layout prep. Returns in_maps for run_bass_kernel_spmd."""
    G, NT = e // 128, s // 128
    Wh = W_attn[:, :d].astype(np.float64)
    Wc = W_attn[:, d:].astype(np.float64)
    # Fold h_proj into the context (exact): delta_b = Wc^T (Wc Wc^T)^-1 hp_b
    # gives Wc (x + delta_b) = c_proj + h_proj_b.
    hp = hidden.astype(np.float64) @ Wh.T + b_attn.astype(np.float64)  # [B, d]
    alpha = np.linalg.solve(Wc @ Wc.T, hp.T)                           # [d, B]
    delta = (Wc.T @ alpha).T.astype(np.float32)                        # [B, e]

    # wcT[p, g, :] = Wc[:, g*128+p]  (moving operand rows = e within chunk)
    wcT = np.ascontiguousarray(
        np.asarray(W_attn[:, d:]).T.reshape(G, 128, d).transpose(1, 0, 2)
    ).astype(np.float16)
    vbc = np.ascontiguousarray(
        np.broadcast_to(v.astype(np.float16), (128, d)))
    eye = np.eye(128, dtype=np.float32)

    in_maps = []
    for i in range(ncores):
        sl = slice(i * bl, (i + 1) * bl)
        ctxT = (np.ascontiguousarray(context[sl].transpose(0, 2, 1))
                + delta[sl][:, :, None]).astype(np.float16)
        # maskT[p, b*NT + t] = mask[b, t*128 + p]
        mT = mask[sl].reshape(bl, NT, 128).transpose(2, 0, 1)
        maskT = np.ascontiguousarray(
            mT.reshape(128, bl * NT)).astype(np.float32)
        in_maps.append({
            "ctxT": ctxT,
            "wcT": wcT,
            "vb": vbc,
            "eye": eye,
            "maskT": maskT,
        })
    return in_maps


_CACHE = {}


def _ensure_ntff_hook_importable():
    """bass_utils' axon trace path imports antenv.axon_hooks, which this
    container's antenv stub lacks. Provide it (with the real ctypes hook when
    available) so BASS_TRACE=1 in the environment can't crash the run."""
    import sys as _sys
    import types as _types

    try:
        import antenv.axon_hooks  # noqa: F401
        return
    except ImportError:
        pass
    mod = _types.ModuleType("antenv.axon_hooks")
    mod._hook = None
    mod.set_axon_ntff_profile_hook = lambda h: setattr(mod, "_hook", h)
    mod.get_axon_ntff_profile_hook = lambda: mod._hook
    _sys.modules["antenv.axon_hooks"] = mod
    try:
        import antenv
        antenv.axon_hooks = mod
        from trn_agent_boot.trn_boot import _ntff_profile_via_ctypes
        mod._hook = _ntff_profile_via_ctypes("/opt/axon/libaxon_pjrt.so")
    except Exception:
        pass


def kernel(hidden, context, mask, W_attn, b_attn, v):
    _ensure_ntff_hook_importable()
    hidden = np.asarray(hidden, dtype=np.float32)
    context = np.asarray(context, dtype=np.float32)
    mask = np.asarray(mask)
    W_attn = np.asarray(W_attn, dtype=np.float32)
    b_attn = np.asarray(b_attn, dtype=np.float32)
    v = np.asarray(v, dtype=np.float32)
    if "nc" not in _CACHE:
        _CACHE["nc"] = build_graph()
    nc = _CACHE["nc"]
    in_maps = shard_inputs(hidden, context, mask, W_attn, b_attn, v)
    res = run_bass_kernel_spmd(nc, in_maps, core_ids=list(range(NCORES)))
    out = np.concatenate([r["out"] for r in res.results], axis=0)
    return out.astype(np.float32)


# revision 49
# speedup vs baseline: 1.0043x; 1.0043x over previous
"""Trainium2 Bass kernel for Bahdanau-style attention scoring.

Reference computation (per batch b):
    h_proj = hidden @ Wh.T + b_attn                  # [D]
    c_proj[s] = Wc @ context[b, s]                   # [S, D]
    scores[s] = v . tanh(h_proj + c_proj[s])         # [S]
    out[b] = softmax(where(mask==0, -inf, scores))   # [S]

Strategy: data-parallel over batch B across 8 NeuronCores (4 batches/core).
The roofline is the c_proj matmul: 1024 [128x128]x[128x512] fp16 matmuls per
core (~222us at the ~216ns/MM N=512 pace). The context is shipped to device
DRAM pre-cast to fp16 (32 MiB/core, ~100us of DMA — same values the SWDGE
cast-on-DMA datapath would produce, at half the HBM traffic), so the
TensorE matmul stream is the single roofline and everything else must stay
off it.

So unlike the usual [d, s] layout, c_proj is computed TRANSPOSED, [s, d]:
the context tile [e=128, s=128] is the stationary operand and WcT [e=128,
d=512] is the moving operand, giving PSUM tiles [s=128, d=512]. With d on
the free axis the v-dot after tanh is a free-axis mul+reduce on VectorE
instead of 128 extra TensorE mat-vec matmuls (which cost ~28us of PE in the
[d, s] layout).

h_proj is folded into the context on the host, exactly: Wc has full row
rank, so delta_b = Wc^T (Wc Wc^T)^{-1} h_proj_b satisfies
Wc (x + delta_b) = c_proj + h_proj_b, and the shard prep adds delta_b[e]
to batch b's context rows. PSUM then holds tanh's full argument directly
(no per-tile broadcast bias add on the [s, d] free axis, which VectorE
would otherwise have to do at fp32 pace).

Per (b, s-tile of 128):
  - 8 accumulating matmuls (e-chunks) -> PSUM y.T [s=128, d=512]
  - ScalarE tanh (PSUM -> SBUF fp16)
  - VectorE (sim * v_bcast) at fp16 2x pace, then free-axis reduce_sum ->
    one column of the per-batch scores tile [128, 32]  (s = tile*128 + p)
Per b (deferred one s-window so TensorE never waits on the chain):
  - ScalarE exp (no max subtraction: |scores| < ~35, far under f32 range),
    VectorE mask-multiply and row-sum, TensorE ones-matmul replicates the
    total over all partitions, VectorE reciprocal, TensorE transpose to
    [32, 128] row-major, VectorE scale-on-copy, DMA out.

DMA queues: gpsimd/SWDGE carries the big context loads, sync/scalar HWDGE
split the wcT preamble between them and carry the outputs. While the ~2MB
preamble (wcT + window 0) fills, ~72 small junk warm-up matmuls bring the PE HAM
clock-gate to 8/8, and window 0's contraction is swept g-outermost across
all four PSUM banks so each matmul only needs the (wcT, ctx) chunk pair
that has already arrived — the PE computes through the fill instead of
waiting for it.
"""

import numpy as np

import concourse.bacc as bacc
import concourse.mybir as mybir
from concourse.tile import TileContext
from concourse.bass_utils import run_bass_kernel_spmd

B, S, E, D = 32, 4096, 1024, 512
NCORES = 8
BL = B // NCORES  # batches per core

F32 = mybir.dt.float32
F16 = mybir.dt.float16


def build_graph(bl=BL, s=S, e=E, d=D, ncores=NCORES):
    """Build the per-core Bass graph. All cores run the same graph (SPMD)."""
    G = e // 128        # e-chunks (contraction passes per tile)
    SW = 512            # s-window per context DMA (4 s-tiles)
    NSW = s // SW       # s-windows per batch
    TPW = SW // 128     # s-tiles per window
    NT = s // 128       # s-tiles per batch (scores columns)
    AF = mybir.ActivationFunctionType

    nc = bacc.Bacc("TRN2", target_bir_lowering=False, debug=False,
                   num_devices=ncores)

    ctxT = nc.dram_tensor("ctxT", [bl, e, s], F16, kind="ExternalInput")
    wcT = nc.dram_tensor("wcT", [128, G, d], F16, kind="ExternalInput")
    vb = nc.dram_tensor("vb", [128, d], F16, kind="ExternalInput")
    eye = nc.dram_tensor("eye", [128, 128], F32, kind="ExternalInput")
    maskT = nc.dram_tensor("maskT", [128, bl * NT], F32, kind="ExternalInput")
    out = nc.dram_tensor("out", [bl, s], F32, kind="ExternalOutput")

    ctx_r = ctxT.ap().rearrange("b (g p) s -> b p g s", p=128)
    out_r = out.ap().rearrange("b (t x) -> b t x", x=128)

    with TileContext(nc) as tc:
        with (
            tc.tile_pool(name="const", bufs=1) as cpool,
            tc.tile_pool(name="ctx", bufs=6) as ctx_pool,
            tc.tile_pool(name="sim", bufs=4) as sim_pool,
            tc.tile_pool(name="prod", bufs=4) as prod_pool,
            tc.tile_pool(name="sc", bufs=2) as sc_pool,
            tc.tile_pool(name="small", bufs=2) as small_pool,
            tc.tile_pool(name="pc", bufs=5, space="PSUM") as pc_pool,
            tc.tile_pool(name="tail", bufs=1, space="PSUM") as tail_pool,
            tc.tile_pool(name="warm", bufs=1, space="PSUM") as warm_pool,
        ):
            # ---- constants / preamble ------------------------------------
            # per-g wcT chunks alternating between the two HWDGE queues:
            # g0 lands first so the first matmul group can start while the
            # later chunks stream in behind it.
            wct_sb = cpool.tile([128, G, d], F16, tag="wct")
            h = G // 2
            for g in range(0, G, 2):
                nc.scalar.dma_start(out=wct_sb[:, g, :], in_=wcT.ap()[:, g, :])
                nc.sync.dma_start(out=wct_sb[:, g + 1, :],
                                  in_=wcT.ap()[:, g + 1, :])
            ones128 = cpool.tile([128, 128], F32, tag="ones128")
            nc.vector.memset(ones128[:], 1.0)
            vb_sb = cpool.tile([128, d], F16, tag="vb")
            nc.sync.dma_start(out=vb_sb[:], in_=vb.ap())

            # eye/maskT are not needed until the first softmax tail
            # (~70us in); declared here, loaded after window 1 is queued on
            # the gpsimd queue so the HWDGE rings stay clear for wcT, whose
            # last chunk gates group 0.
            eye_sb = cpool.tile([128, 128], F32, tag="eye")
            maskt_sb = cpool.tile([128, bl * NT], F32, tag="maskt")

            def load_tail_consts():
                nc.gpsimd.dma_start(out=eye_sb[:], in_=eye.ap())
                nc.gpsimd.dma_start(out=maskt_sb[:], in_=maskT.ap())

            # PE warm-up: junk matmuls while the first context chunks
            # stream in, so the HAM clock-gate is warming before the real
            # stream starts (cold matmuls run at half rate).
            junk = cpool.tile([128, 128], F16, tag="junk")
            nc.vector.memset(junk[:], 0.0)
            warm_ps = warm_pool.tile([128, 128], F32, tag="warm")
            for _ in range(72):
                nc.tensor.matmul(warm_ps[:, 0:64], lhsT=junk[:],
                                 rhs=junk[:, 0:64], start=True, stop=True)

            # ---- main loop ------------------------------------------------
            pend = None  # deferred softmax tail of the previous batch

            def tail(b, scores, part=None):
                # scores [128, NT]: s = t*128 + p.  The mask is applied as
                # an additive -1e4 bias before exp (exp underflows to an
                # exact 0 for masked slots), so one ScalarE exp with
                # accum_out yields both the masked exp row and its row sum.
                rsum = small_pool.tile([128, 1], F32, tag="rsum")
                if part is None:
                    sb = small_pool.tile([128, NT], F32, tag="sb")
                    nc.vector.tensor_add(sb[:], scores[:],
                                         maskt_sb[:, b * NT:(b + 1) * NT])
                    erow = small_pool.tile([128, NT], F32, tag="erow")
                    nc.scalar.activation(erow[:], sb[:], AF.Exp,
                                         accum_out=rsum[:])
                else:
                    # columns 0:cc were exp'd/summed a window ago
                    erow, sb, rsa = part
                    cc = (NSW - 1) * TPW
                    nc.vector.tensor_add(
                        sb[:, cc:NT], scores[:, cc:NT],
                        maskt_sb[:, b * NT + cc:(b + 1) * NT])
                    rsb = small_pool.tile([128, 1], F32, tag="rsb")
                    nc.scalar.activation(erow[:, cc:NT], sb[:, cc:NT],
                                         AF.Exp, accum_out=rsb[:])
                    nc.vector.tensor_add(rsum[:], rsa[:], rsb[:])
                # total over partitions, replicated to every partition
                tot_ps = tail_pool.tile([128, 1], F32, tag="tot")
                nc.tensor.matmul(tot_ps[:], lhsT=ones128[:], rhs=rsum[:],
                                 start=True, stop=True)
                rec = small_pool.tile([128, 1], F32, tag="rec")
                nc.vector.reciprocal(rec[:], tot_ps[:])
                # transpose to row-major [t=32, x=128] and scale on the copy
                et_ps = tail_pool.tile([NT, 128], F32, tag="et")
                nc.tensor.transpose(et_ps[:], erow[:], eye_sb[:])
                orow = small_pool.tile([NT, 128], F32, tag="orow")
                nc.vector.tensor_scalar_mul(orow[:], et_ps[:], rec[0:NT, :])
                hh = NT // 2
                nc.sync.dma_start(out=out_r[b, 0:hh], in_=orow[0:hh, :])
                nc.scalar.dma_start(out=out_r[b, hh:NT], in_=orow[hh:NT, :])

            def chain(pc, scores, st, split=False):
                # tanh -> (sim * v) -> free-axis sum -> scores column
                sim = sim_pool.tile([128, 512], F16, tag="sim")
                prod = prod_pool.tile([128, 512], F16, tag="prod")
                if split:
                    # halve the chain so ScalarE/VectorE pipeline the
                    # fully-exposed serial tail after the last matmul group
                    s2 = small_pool.tile([128, 2], F32, tag="s2")
                    for q in range(2):
                        cut = slice(q * 256, (q + 1) * 256)
                        nc.scalar.activation(sim[:, cut], pc[:, cut], AF.Tanh)
                        nc.vector.tensor_mul(prod[:, cut], sim[:, cut],
                                             vb_sb[:, cut])
                        nc.vector.reduce_sum(s2[:, q:q + 1], prod[:, cut],
                                             axis=mybir.AxisListType.X)
                    nc.vector.tensor_add(scores[:, st:st + 1],
                                         s2[:, 0:1], s2[:, 1:2])
                else:
                    nc.scalar.activation(sim[:], pc[:], AF.Tanh)
                    nc.vector.tensor_mul(prod[:], sim[:], vb_sb[:])
                    nc.vector.reduce_sum(scores[:, st:st + 1], prod[:],
                                         axis=mybir.AxisListType.X)

            part = None  # pre-computed exp/mask/rowsum of the last batch

            for b in range(bl):
                scores = sc_pool.tile([128, NT], F32, tag="scores")
                for sw in range(NSW):
                    ctx_slice = ctx_r[b, :, :, sw * SW:(sw + 1) * SW]
                    ctx_t = ctx_pool.tile([128, G, SW], F16, tag="ctx")
                    if b == 0 and sw <= 1:
                        # pipe-fill: fine-grained chunk DMAs, and the
                        # contraction swept g-outer across all 4 PSUM banks
                        # — each matmul needs only the (wcT, ctx) chunk
                        # pair g, so the PE computes through the whole 3MB
                        # preamble fill instead of waiting for it.
                        if sw == 0:
                            for g in range(G):
                                nc.gpsimd.dma_start(
                                    out=ctx_t[:, g, :],
                                    in_=ctx_slice[:, g, :])
                        else:
                            nc.gpsimd.dma_start(
                                out=ctx_t[:, 0:h, :], in_=ctx_slice[:, 0:h, :])
                            nc.gpsimd.dma_start(
                                out=ctx_t[:, h:G, :], in_=ctx_slice[:, h:G, :])
                        pcs = [pc_pool.tile([128, 512], F32, tag="pc",
                                            name=f"pc{sw}_{t}")
                               for t in range(TPW)]
                        for g in range(G):
                            for t in range(TPW):
                                nc.tensor.matmul(
                                    pcs[t][:],
                                    lhsT=ctx_t[:, g, t * 128:(t + 1) * 128],
                                    rhs=wct_sb[:, g, :],
                                    start=(g == 0), stop=(g == G - 1),
                                )
                        for t in range(TPW):
                            chain(pcs[t], scores, sw * TPW + t)
                        continue
                    else:
                        # 1MB fp16 read per window
                        nc.gpsimd.dma_start(out=ctx_t[:], in_=ctx_slice)
                        if b == 0 and sw == 2:
                            load_tail_consts()
                    for t in range(TPW):
                        st = sw * TPW + t
                        if b == bl - 1 and st == NT - 1:
                            # final tile: two d-half matmul groups, so the
                            # first half's tanh/dot chain runs while the
                            # second half's matmuls are still on the PE —
                            # this chain is the only fully-exposed one.
                            s2 = small_pool.tile([128, 2], F32, tag="s2")
                            for q in range(2):
                                cut = slice(q * 256, (q + 1) * 256)
                                pch = pc_pool.tile([128, 512], F32,
                                                   tag="pc", name=f"pch{q}")
                                for g in range(G):
                                    nc.tensor.matmul(
                                        pch[:, cut],
                                        lhsT=ctx_t[:, g,
                                                   t * 128:(t + 1) * 128],
                                        rhs=wct_sb[:, g, cut],
                                        start=(g == 0), stop=(g == G - 1),
                                    )
                                sim = sim_pool.tile([128, 512], F16,
                                                    tag="sim",
                                                    name=f"simh{q}")
                                nc.scalar.activation(sim[:, cut],
                                                     pch[:, cut], AF.Tanh)
                                prod = prod_pool.tile([128, 512], F16,
                                                      tag="prod",
                                                      name=f"prodh{q}")
                                nc.vector.tensor_mul(prod[:, cut],
                                                     sim[:, cut],
                                                     vb_sb[:, cut])
                                nc.vector.reduce_sum(
                                    s2[:, q:q + 1], prod[:, cut],
                                    axis=mybir.AxisListType.X)
                            nc.vector.tensor_add(scores[:, st:st + 1],
                                                 s2[:, 0:1], s2[:, 1:2])
                            continue
                        pc = pc_pool.tile([128, 512], F32, tag="pc")
                        for g in range(G):
                            nc.tensor.matmul(
                                pc[:],
                                lhsT=ctx_t[:, g, t * 128:(t + 1) * 128],
                                rhs=wct_sb[:, g, :],
                                start=(g == 0), stop=(g == G - 1),
                            )
                        chain(pc, scores, st)
                    # batch b-1's softmax tail goes out after batch b's
                    # first window is queued, so TensorE's tiny tail ops
                    # never make it wait on the Scalar/Vector chain.
                    if sw == 0 and pend is not None:
                        tail(*pend)
                        pend = None
                    if b == bl - 1 and sw == NSW - 2:
                        # pre-exp/rowsum the 28 finished columns of the
                        # final batch so its exposed tail chain is short
                        erow = small_pool.tile([128, NT], F32, tag="erow")
                        sb = small_pool.tile([128, NT], F32, tag="sb")
                        rsa = small_pool.tile([128, 1], F32, tag="rsa")
                        cc = (NSW - 1) * TPW
                        nc.vector.tensor_add(sb[:, 0:cc], scores[:, 0:cc],
                                             maskt_sb[:, b * NT:b * NT + cc])
                        nc.scalar.activation(erow[:, 0:cc], sb[:, 0:cc],
                                             AF.Exp, accum_out=rsa[:])
                        part = (erow, sb, rsa)
                pend = (b, scores)

            tail(*pend, part=part)

    nc.compile()
    return nc


def shard_inputs(hidden, context, mask, W_attn, b_attn, v,
                 bl=BL, s=S, e=E, d=D, ncores=NCORES):
    """Host-side shard + layout prep. Returns in_maps for run_bass_kernel_spmd."""
    G, NT = e // 128, s // 128
    Wh = W_attn[:, :d].astype(np.float64)
    Wc = W_attn[:, d:].astype(np.float64)
    # Fold h_proj into the context (exact): delta_b = Wc^T (Wc Wc^T)^-1 hp_b
    # gives Wc (x + delta_b) = c_proj + h_proj_b.
    hp = hidden.astype(np.float64) @ Wh.T + b_attn.astype(np.float64)  # [B, d]
    alpha = np.linalg.solve(Wc @ Wc.T, hp.T)                           # [d, B]
    delta = (Wc.T @ alpha).T.astype(np.float32)                        # [B, e]

    # wcT[p, g, :] = Wc[:, g*128+p]  (moving operand rows = e within chunk)
    wcT = np.ascontiguousarray(
        np.asarray(W_attn[:, d:]).T.reshape(G, 128, d).transpose(1, 0, 2)
    ).astype(np.float16)
    vbc = np.ascontiguousarray(
        np.broadcast_to(v.astype(np.float16), (128, d)))
    eye = np.eye(128, dtype=np.float32)

    in_maps = []
    for i in range(ncores):
        sl = slice(i * bl, (i + 1) * bl)
        ctxT = (np.ascontiguousarray(context[sl].transpose(0, 2, 1))
                + delta[sl][:, :, None]).astype(np.float16)
        # maskT[p, b*NT + t] = additive bias for mask[b, t*128 + p]:
        # 0 where unmasked, -1e4 where masked (exp underflows to exact 0)
        mT = mask[sl].reshape(bl, NT, 128).transpose(2, 0, 1)
        maskT = np.ascontiguousarray(np.where(
            mT.reshape(128, bl * NT) == 0, -1.0e4, 0.0)).astype(np.float32)
        in_maps.append({
            "ctxT": ctxT,
            "wcT": wcT,
            "vb": vbc,
            "eye": eye,
            "maskT": maskT,
        })
    return in_maps


_CACHE = {}


def _ensure_ntff_hook_importable():
    """bass_utils' axon trace path imports antenv.axon_hooks, which this
    container's antenv stub lacks. Provide it (with the real ctypes hook when
    available) so BASS_TRACE=1 in the environment can't crash the run."""
    import sys as _sys
    import types as _types

    try:
        import antenv.axon_hooks  # noqa: F401
        return
    except ImportError:
        pass
    mod = _types.ModuleType("antenv.axon_hooks")
    mod._hook = None
    mod.set_axon_ntff_profile_hook = lambda h: setattr(mod, "_hook", h)
    mod.get_axon_ntff_profile_hook = lambda: mod._hook
    _sys.modules["antenv.axon_hooks"] = mod
    try:
        import antenv
        antenv.axon_hooks = mod
        from trn_agent_boot.trn_boot import _ntff_profile_via_ctypes
        mod._hook = _ntff_profile_via_ctypes("/opt/axon/libaxon_pjrt.so")
    except Exception:
        pass


def kernel(hidden, context, mask, W_attn, b_attn, v):
    _ensure_ntff_hook_importable()
    hidden = np.asarray(hidden, dtype=np.float32)
    context = np.asarray(context, dtype=np.float32)
    mask = np.asarray(mask)
    W_attn = np.asarray(W_attn, dtype=np.float32)
    b_attn = np.asarray(b_attn, dtype=np.float32)
    v = np.asarray(v, dtype=np.float32)
    if "nc" not in _CACHE:
        _CACHE["nc"] = build_graph()
    nc = _CACHE["nc"]
    in_maps = shard_inputs(hidden, context, mask, W_attn, b_attn, v)
    res = run_bass_kernel_spmd(nc, in_maps, core_ids=list(range(NCORES)))
    out = np.concatenate([r["out"] for r in res.results], axis=0)
    return out.astype(np.float32)
